# revision 37
# baseline (speedup 1.0000x reference)
"""Trainium2 Bass kernel for Ag3LESModel (nn_Ag3LESModel_52158082842739).

Computes, for a 3-atom system:
  raw_q  = MLP_c(features)[:, 0]                  (16->32->32->1, SiLU)
  latent = raw_q + (charge_state - sum(raw_q))/3
  E_lr   = sum_{i<j} latent_i latent_j / |p_i - p_j|
  E_sr   = sum(MLP_s(features)[:, 0])             (16->32->32->1, SiLU)
  returns (E_lr + E_sr, latent)

Device strategy (single NeuronCore, replicated across the 8 cores):
  - Both MLPs fused into one stack: layer1/2 run as single matmuls over a
    64-wide concatenated hidden dim; layer 3 splits into two (1,3) matmuls.
  - cb3 cancels out of latent_q analytically, so it is dropped. sb3 is
    folded into the E_sr matmul via an extra ones row (lhsT = [sw3; sb3]).
  - 1/r via Quake rsqrt (int bit trick + 2 Newton steps) on the vector
    engine: avoids loading the ACT sqrt table set (only Silu's set loads).
  - Final energy assembled with one fused tensor_tensor_reduce.
All inputs are packed host-side into one (65,142) f32 tile -> single DMA in,
single (1,4) DMA out: [E, q0, q1, q2].
"""

import os
import sys

import numpy as np

if "/opt/trn_rl_repo" not in sys.path:
    sys.path.insert(0, "/opt/trn_rl_repo")

N = 3
_II = (0, 0, 1)
_JJ = (1, 2, 2)

# f32 packed-tile column layout (biases, pair geometry, scalars)
_F = 11
_C_B1 = 0      # col 0, rows 0:64 : [cb1; sb1]
_C_B2 = 1      # col 1, rows 0:64 : [cb2; sb2]
_C_PA = 2      # cols 2:5, rows 0:3 : positions[II].T
_C_PB = 5      # cols 5:8, rows 0:3 : positions[JJ].T
_C_ON = 8      # col 8, rows 0:3 : ones
_C_C3 = 9      # col 9, row 0 : charge_state / 3
_C_SB3 = 10    # col 10, row 0 : 3 * sb3

# bf16 packed-tile column layout (matmul weights + features)
_FB = 132
_B_W2 = 0      # cols 0:64, rows 0:64 : blockdiag(cw2, sw2)
_B_W3 = 64     # col 64: rows 0:32 cw3, rows 32:64 sw3
_B_W1 = 65     # cols 65:129, rows 0:16 : [cw1 | sw1]
_B_XT = 129    # cols 129:132, rows 0:16 : features.T

_NC = None
_DRAIN_PATCHED = False


def _patch_drain_wait_split(tile, mybir, max_waits=1):
    # Replace Tile's kernel tail. Stock tail = drain + all-engine barrier +
    # semaphore clear + second all-engine barrier: the two EVSEM butterflies
    # cost ~7us on silicon. Here: a chain of 1-wait drains on the sync engine
    # (the drain encoding holds only one wait; the stock 5-wait drain fails
    # codegen), the last of which bumps a fresh 'done' semaphore; gpsimd
    # waits on it, then resets DMA queues and clears all semaphores
    # (including 'done'), so the NEFF stays re-executable. Once the drain
    # chain has retired, every engine has passed its last semaphore wait,
    # so the barrier-free clear cannot strand a waiter.
    global _DRAIN_PATCHED
    if _DRAIN_PATCHED:
        return
    _DRAIN_PATCHED = True
    orig = tile.TileContext._drain_and_barrier

    def patched(self, tick_clock, wait_clock):
        from concourse.vector_clock import ScopedClock

        nc = self.nc
        drain_inst = nc.sync.drain()
        wait_clock.add_sem_waits(
            drain_inst.ins, ScopedClock({None: tick_clock.global_clock})
        )
        si = drain_inst.ins.sync_info
        waits = list(si.on_wait) if si is not None else []
        upds = list(si.on_update) if si is not None else []
        chain = [drain_inst]
        if len(waits) > max_waits:
            drain_inst.ins.sync_info = mybir.SyncInfo(
                on_wait=waits[:max_waits], on_update=upds
            )
            rest = waits[max_waits:]
            while rest:
                extra = nc.sync.drain()
                extra.ins.sync_info = mybir.SyncInfo(
                    on_wait=rest[:max_waits], on_update=[]
                )
                chain.append(extra)
                rest = rest[max_waits:]
        done = nc.alloc_semaphore("tail_done")
        chain[-1].then_inc(done, 1)

        assert self.sems is not None
        popped = nc._tile_sem_poison_stack.pop()
        assert popped is self._sem_poison
        nc.gpsimd.wait_ge(done, 1)
        nc.clear_and_free_semaphores(
            list(self.sems.allocated().values()) + [done]
        )

    patched._orig = orig
    tile.TileContext._drain_and_barrier = patched


def _build_nc(silu_via_sigmoid=False):
    # silu_via_sigmoid: CoreSim has no Silu table; build silu(x) = x*sigmoid(x)
    # out of Sigmoid + vector ops for the sim gate. HW uses Silu directly.
    import concourse.bass as bass
    import concourse.mybir as mybir
    import concourse.tile as tile

    _patch_drain_wait_split(tile, mybir)

    fp32 = mybir.dt.float32
    bf16 = mybir.dt.bfloat16
    i32 = mybir.dt.int32
    AF = mybir.ActivationFunctionType
    OP = mybir.AluOpType
    AX = mybir.AxisListType

    nc = bass.Bass("TRN2", target_bir_lowering=False, debug=False)
    # Drop the const-pool memsets Bass.__init__ emits (0.0 / 1.0 / bf16-1.0 /
    # u8-127): nothing in this kernel reads them, and they burn ~400ns of
    # gpsimd time at the head of the measured window.
    _insts = nc.m.functions[0].blocks[0].instructions
    for _i in range(len(_insts) - 1, -1, -1):
        if type(_insts[_i]).__name__ == "InstMemset":
            del _insts[_i]
    pk = nc.declare_dram_parameter("pack", [64, _F], fp32, isOutput=False)
    pkb = nc.declare_dram_parameter("packb", [64, _FB], bf16, isOutput=False)
    od = nc.declare_dram_parameter("out", [1, 4], fp32, isOutput=True)

    with tile.TileContext(nc) as tc:
        with (
            tc.tile_pool(name="sb", bufs=1) as sb,
            tc.tile_pool(name="ps", bufs=1, space="PSUM") as ps,
        ):
            PB16 = sb.tile([64, _FB], bf16)
            nc.sync.dma_start(out=PB16[:, :], in_=pkb[:, :])
            P = sb.tile([64, _F], fp32)
            nc.sync.dma_start(out=P[:, :], in_=pk[:, :])

            W2 = PB16[0:64, _B_W2:_B_W2 + 64]
            w3q = PB16[0:32, _B_W3:_B_W3 + 1]
            w3s = PB16[32:64, _B_W3:_B_W3 + 1]
            W1 = PB16[0:16, _B_W1:_B_W1 + 64]
            XT = PB16[0:16, _B_XT:_B_XT + 3]
            B1 = P[0:64, _C_B1:_C_B1 + 1]
            B2 = P[0:64, _C_B2:_C_B2 + 1]
            PA = P[0:3, _C_PA:_C_PA + 3]
            PB = P[0:3, _C_PB:_C_PB + 3]
            ON3 = P[0:3, _C_ON:_C_ON + 1]
            C3 = P[0:1, _C_C3:_C_C3 + 1]
            SB3 = P[0:1, _C_SB3:_C_SB3 + 1]

            # ACT primer: the ACTIVATE encoding supports a single sync wait,
            # but silu1 would need two (PE matmul + DMA'd bias tile). This
            # op makes ACT observe the input-DMA semaphore first, so silu1
            # only waits on PE.
            AJ = sb.tile([1, 1], fp32)
            nc.scalar.activation(AJ[:, :], C3, AF.Copy)
            # PE primer, same story: the r2 matmul reads ON3 (f32-pack DMA)
            # and DD (vector engine) — prime PE on the f32-pack semaphore so
            # r2 only waits on the vector engine.
            PJ = ps.tile([1, 1], fp32)
            nc.tensor.matmul(PJ[:, :], ON3, ON3, start=True, stop=True)

            # ---- pair-distance branch: rinv = 1/|p_i - p_j| per pair ----
            D = sb.tile([3, 3], fp32)
            nc.vector.tensor_sub(D[:, :], PA, PB)
            DD = sb.tile([3, 3], fp32)
            nc.vector.tensor_mul(DD[:, :], D[:, :], D[:, :])
            R2p = ps.tile([1, 3], fp32)
            nc.tensor.matmul(R2p[:, :], ON3, DD[:, :], start=True, stop=True)
            # Quake rsqrt straight off PSUM: y0_bits = 0x5f3759df - (x>>1)
            #                              = ((x>>1) ^ -1) + 0x5f3759e0
            Y = sb.tile([1, 3], fp32)
            nc.vector.tensor_scalar(
                Y.bitcast(i32)[:, :], R2p.bitcast(i32)[:, :],
                1, -1, OP.arith_shift_right, OP.bitwise_xor,
            )
            nc.vector.tensor_scalar(
                Y.bitcast(i32)[:, :], Y.bitcast(i32)[:, :],
                0x5F3759E0, None, OP.add,
            )
            # Newton: y <- y * (1.5 - 0.5 x y^2); ~1.8e-3 rel after one step
            T = sb.tile([1, 3], fp32)
            nc.vector.tensor_mul(T[:, :], Y[:, :], Y[:, :])
            nc.vector.tensor_mul(T[:, :], T[:, :], R2p[:, :])
            nc.vector.tensor_scalar(T[:, :], T[:, :], -0.5, 1.5, OP.mult, OP.add)
            nc.vector.tensor_mul(Y[:, :], Y[:, :], T[:, :])

            # ---- fused dual MLP (hidden dims stacked: charge 0:32, sr 32:64) ----
            def silu(out_ap, in_psum, bias_ap, rows):
                if not silu_via_sigmoid:
                    nc.scalar.activation(out_ap, in_psum, AF.Silu, bias=bias_ap, scale=1.0)
                    return
                X1 = sb.tile([rows, 3], fp32, tag=f"x{rows}")
                nc.vector.tensor_scalar(X1[:, :], in_psum, bias_ap, None, OP.add)
                SG = sb.tile([rows, 3], fp32, tag=f"s{rows}")
                nc.scalar.activation(SG[:, :], in_psum, AF.Sigmoid, bias=bias_ap, scale=1.0)
                nc.vector.tensor_mul(out_ap, X1[:, :], SG[:, :])

            M1 = ps.tile([64, 3], fp32)
            nc.tensor.matmul(M1[:, :], W1, XT, start=True, stop=True)
            H1 = sb.tile([64, 3], bf16)
            silu(H1[:, :], M1[:, :], B1, 64)
            M2 = ps.tile([64, 3], fp32)
            nc.tensor.matmul(M2[:, :], W2, H1[:, :], start=True, stop=True)
            H2 = sb.tile([64, 3], bf16)
            silu(H2[:, :], M2[:, :], B2, 64)
            T3q = ps.tile([1, 3], fp32)  # per-atom raw_q (minus cb3, which cancels)
            nc.tensor.matmul(T3q[:, :], w3q, H2[0:32, :], start=True, stop=True)
            T3s = ps.tile([1, 3], fp32)  # per-atom short-range E (minus sb3)
            nc.tensor.matmul(T3s[:, :], w3s, H2[32:64, :], start=True, stop=True)

            # ---- charge correction + energy assembly ----
            # SR = sum_i T3s_i (3*sb3 is folded into the final add)
            SR = sb.tile([1, 1], fp32)
            nc.vector.reduce_sum(SR[:, :], T3s[:, :], axis=AX.X)
            # corr = -(sum T3q)/3 + C/3 in ONE fused op: out = T3q * -1/3,
            # accum = reduce_add(out) then + C/3 (scalar2 post-reduce)
            CR = sb.tile([1, 1], fp32)
            CAj = sb.tile([1, 3], fp32)
            nc.vector.tensor_scalar(
                CAj[:, :], T3q[:, :], -1.0 / 3.0, C3, OP.mult, OP.add,
                accum_out=CR[0:1, 0:1],
            )
            OUT = sb.tile([1, 4], fp32)
            nc.vector.tensor_scalar(
                OUT[0:1, 1:4], T3q[:, :], CR[0:1, 0:1], None, OP.add,
            )
            PQ = sb.tile([1, 3], fp32)
            nc.vector.tensor_scalar(
                PQ[0:1, 0:2], OUT[0:1, 2:4], OUT[0:1, 1:2], None, OP.mult,
            )
            nc.vector.tensor_mul(PQ[0:1, 2:3], OUT[0:1, 2:3], OUT[0:1, 3:4])
            EP3 = sb.tile([1, 3], fp32)
            nc.vector.tensor_mul(EP3[:, :], PQ[:, :], Y[:, :])
            EE = sb.tile([1, 1], fp32)
            nc.vector.reduce_sum(EE[:, :], EP3[:, :], axis=AX.X)
            nc.vector.tensor_scalar(
                OUT[0:1, 0:1], EE[:, :], SR[0:1, 0:1], SB3, OP.add, OP.add,
            )
            nc.sync.dma_start(out=od[:, :], in_=OUT[:, :])

    return nc


def _get_nc():
    global _NC
    if _NC is None:
        _NC = _build_nc()
    return _NC


def _pack_inputs(inputs):
    import ml_dtypes

    g = lambda k: np.asarray(inputs[k], dtype=np.float32)
    pack = np.zeros((64, _F), dtype=np.float32)
    pack[0:32, _C_B1] = g("cb1")
    pack[32:64, _C_B1] = g("sb1")
    pack[0:32, _C_B2] = g("cb2")
    pack[32:64, _C_B2] = g("sb2")
    pos = g("positions")
    pack[0:3, _C_PA:_C_PA + 3] = pos[list(_II)].T
    pack[0:3, _C_PB:_C_PB + 3] = pos[list(_JJ)].T
    pack[0:3, _C_ON] = 1.0
    pack[0, _C_C3] = float(np.asarray(inputs["charge_state"])) / 3.0
    pack[0, _C_SB3] = 3.0 * g("sb3")[0]

    packb = np.zeros((64, _FB), dtype=np.float32)
    packb[0:32, _B_W2:_B_W2 + 32] = g("cw2")
    packb[32:64, _B_W2 + 32:_B_W2 + 64] = g("sw2")
    packb[0:32, _B_W3] = g("cw3")[:, 0]
    packb[32:64, _B_W3] = g("sw3")[:, 0]
    packb[0:16, _B_W1:_B_W1 + 32] = g("cw1")
    packb[0:16, _B_W1 + 32:_B_W1 + 64] = g("sw1")
    packb[0:16, _B_XT:_B_XT + 3] = g("features").T
    return pack, packb.astype(ml_dtypes.bfloat16)


def _install_ntff_hook():
    # The image's antenv lacks axon_hooks; inject it and wire the ctypes
    # NTFF profile hook from trn_boot so trace=True yields exec_time_ns.
    try:
        from antenv.axon_hooks import get_axon_ntff_profile_hook  # noqa: F401
        return
    except ImportError:
        pass
    import types

    import antenv
    from trn_agent_boot.trn_boot import _ntff_profile_via_ctypes

    mod = types.ModuleType("antenv.axon_hooks")
    state = {"hook": None}
    mod.set_axon_ntff_profile_hook = lambda h: state.__setitem__("hook", h)
    mod.get_axon_ntff_profile_hook = lambda: state["hook"]
    sys.modules["antenv.axon_hooks"] = mod
    antenv.axon_hooks = mod
    mod.set_axon_ntff_profile_hook(
        _ntff_profile_via_ctypes("/opt/axon/libaxon_pjrt.so")
    )


def run(inputs, trace=False):
    """Run on hardware across 8 cores (replicated). Returns ((E, q), exec_ns)."""
    from concourse.bass_utils import run_bass_kernel_spmd

    if trace:
        _install_ntff_hook()
    nc = _get_nc()
    pack, packb = _pack_inputs(inputs)
    in_maps = [{"pack": pack, "packb": packb} for _ in range(8)]
    res = run_bass_kernel_spmd(nc, in_maps, list(range(8)), trace=trace)
    out = np.asarray(res.results[0]["out"], dtype=np.float32)
    E = np.asarray(out[0, 0], dtype=np.float32)
    q = out[0, 1:4].astype(np.float32)
    return (E, q), res.exec_time_ns


def kernel(**inputs):
    (E, q), _ = run(inputs)
    return (E, q)


# revision 43
# speedup vs baseline: 1.0556x; 1.0556x over previous
"""Trainium2 Bass kernel for Ag3LESModel (nn_Ag3LESModel_52158082842739).

Computes, for a 3-atom system:
  raw_q  = MLP_c(features)[:, 0]                  (16->32->32->1, SiLU)
  latent = raw_q + (charge_state - sum(raw_q))/3
  E_lr   = sum_{i<j} latent_i latent_j / |p_i - p_j|
  E_sr   = sum(MLP_s(features)[:, 0])             (16->32->32->1, SiLU)
  returns (E_lr + E_sr, latent)

Device strategy (single NeuronCore, replicated across the 8 cores):
  - Both MLPs fused into one stack: layer1/2 run as single matmuls over a
    64-wide concatenated hidden dim; layer 3 splits into two (1,3) matmuls.
  - cb3 cancels out of latent_q analytically, so it is dropped. sb3 is
    folded into the E_sr matmul via an extra ones row (lhsT = [sw3; sb3]).
  - 1/r via Quake rsqrt (int bit trick + 2 Newton steps) on the vector
    engine: avoids loading the ACT sqrt table set (only Silu's set loads).
  - Final energy assembled with one fused tensor_tensor_reduce.
All inputs are packed host-side into one (65,142) f32 tile -> single DMA in,
single (1,4) DMA out: [E, q0, q1, q2].
"""

import os
import sys

import numpy as np

if "/opt/trn_rl_repo" not in sys.path:
    sys.path.insert(0, "/opt/trn_rl_repo")

N = 3
_II = (0, 0, 1)
_JJ = (1, 2, 2)

# One f32 packed tile per core: cols 0:11 hold f32 data (biases, pair
# geometry, scalars); cols 11:77 hold a bf16 sub-pack (matmul weights +
# features) stored as raw 32-bit words and bitcast to bf16 on-chip.
_F = 77
_C_B1 = 0      # col 0, rows 0:64 : [cb1; sb1]
_C_B2 = 1      # col 1, rows 0:64 : [cb2; sb2]
_C_PA = 2      # cols 2:5, rows 0:3 : positions[II].T
_C_PB = 5      # cols 5:8, rows 0:3 : positions[JJ].T
_C_ON = 8      # col 8, rows 0:3 : ones
_C_C3 = 9      # col 9, row 0 : charge_state / 3
_C_SB3 = 10    # col 10, row 0 : 3 * sb3
_C_BF = 11     # cols 11:77 : bf16 sub-pack (132 bf16 columns)

# bf16 sub-pack column layout (in bf16 units, offset by 2*_C_BF on chip)
_FB = 132
_B_W2 = 0      # cols 0:64, rows 0:64 : blockdiag(cw2, sw2)
_B_W3 = 64     # col 64: rows 0:32 cw3, rows 32:64 sw3
_B_W1 = 65     # cols 65:129, rows 0:16 : [cw1 | sw1]
_B_XT = 129    # cols 129:132, rows 0:16 : features.T

_NC = None
_DRAIN_PATCHED = False


def _patch_drain_wait_split(tile, mybir, max_waits=1):
    # Replace Tile's kernel tail. Stock tail = drain + all-engine barrier +
    # semaphore clear + second all-engine barrier: the two EVSEM butterflies
    # cost ~7us on silicon. Here: a chain of 1-wait drains on the sync engine
    # (the drain encoding holds only one wait; the stock 5-wait drain fails
    # codegen), the last of which bumps a fresh 'done' semaphore; gpsimd
    # waits on it, then resets DMA queues and clears all semaphores
    # (including 'done'), so the NEFF stays re-executable. Once the drain
    # chain has retired, every engine has passed its last semaphore wait,
    # so the barrier-free clear cannot strand a waiter.
    global _DRAIN_PATCHED
    if _DRAIN_PATCHED:
        return
    _DRAIN_PATCHED = True
    orig = tile.TileContext._drain_and_barrier

    def patched(self, tick_clock, wait_clock):
        from concourse.vector_clock import ScopedClock

        nc = self.nc
        drain_inst = nc.sync.drain()
        wait_clock.add_sem_waits(
            drain_inst.ins, ScopedClock({None: tick_clock.global_clock})
        )
        si = drain_inst.ins.sync_info
        waits = list(si.on_wait) if si is not None else []
        upds = list(si.on_update) if si is not None else []
        chain = [drain_inst]
        if len(waits) > max_waits:
            drain_inst.ins.sync_info = mybir.SyncInfo(
                on_wait=waits[:max_waits], on_update=upds
            )
            rest = waits[max_waits:]
            while rest:
                extra = nc.sync.drain()
                extra.ins.sync_info = mybir.SyncInfo(
                    on_wait=rest[:max_waits], on_update=[]
                )
                chain.append(extra)
                rest = rest[max_waits:]
        done = nc.alloc_semaphore("tail_done")
        chain[-1].then_inc(done, 1)

        assert self.sems is not None
        popped = nc._tile_sem_poison_stack.pop()
        assert popped is self._sem_poison
        nc.gpsimd.wait_ge(done, 1)
        nc.clear_and_free_semaphores(
            list(self.sems.allocated().values()) + [done]
        )

    patched._orig = orig
    tile.TileContext._drain_and_barrier = patched


def _build_nc(silu_via_sigmoid=False):
    # silu_via_sigmoid: CoreSim has no Silu table; build silu(x) = x*sigmoid(x)
    # out of Sigmoid + vector ops for the sim gate. HW uses Silu directly.
    import concourse.bass as bass
    import concourse.mybir as mybir
    import concourse.tile as tile

    _patch_drain_wait_split(tile, mybir)

    fp32 = mybir.dt.float32
    bf16 = mybir.dt.bfloat16
    i32 = mybir.dt.int32
    AF = mybir.ActivationFunctionType
    OP = mybir.AluOpType
    AX = mybir.AxisListType

    nc = bass.Bass("TRN2", target_bir_lowering=False, debug=False)
    # Drop the const-pool memsets Bass.__init__ emits (0.0 / 1.0 / bf16-1.0 /
    # u8-127): nothing in this kernel reads them, and they burn ~400ns of
    # gpsimd time at the head of the measured window.
    _insts = nc.m.functions[0].blocks[0].instructions
    for _i in range(len(_insts) - 1, -1, -1):
        if type(_insts[_i]).__name__ == "InstMemset":
            del _insts[_i]
    pk = nc.declare_dram_parameter("pack", [64, _F], fp32, isOutput=False)
    od = nc.declare_dram_parameter("out", [1, 4], fp32, isOutput=True)

    with tile.TileContext(nc) as tc:
        with (
            tc.tile_pool(name="sb", bufs=1) as sb,
            tc.tile_pool(name="ps", bufs=1, space="PSUM") as ps,
        ):
            P = sb.tile([64, _F], fp32)
            nc.sync.dma_start(out=P[:, :], in_=pk[:, :])

            PV = P.bitcast(bf16)  # (64, 2*_F) bf16 view of the same bytes
            _o = 2 * _C_BF
            W2 = PV[0:64, _o + _B_W2:_o + _B_W2 + 64]
            w3q = PV[0:32, _o + _B_W3:_o + _B_W3 + 1]
            w3s = PV[32:64, _o + _B_W3:_o + _B_W3 + 1]
            W1 = PV[0:16, _o + _B_W1:_o + _B_W1 + 64]
            XT = PV[0:16, _o + _B_XT:_o + _B_XT + 3]
            B1 = P[0:64, _C_B1:_C_B1 + 1]
            B2 = P[0:64, _C_B2:_C_B2 + 1]
            PA = P[0:3, _C_PA:_C_PA + 3]
            PB = P[0:3, _C_PB:_C_PB + 3]
            ON3 = P[0:3, _C_ON:_C_ON + 1]
            C3 = P[0:1, _C_C3:_C_C3 + 1]
            SB3 = P[0:1, _C_SB3:_C_SB3 + 1]

            # ACT primer: the ACTIVATE encoding supports a single sync wait,
            # but silu1 would need two (PE matmul + DMA'd bias tile). This
            # op makes ACT observe the input-DMA semaphore first, so silu1
            # only waits on PE.
            AJ = sb.tile([1, 1], fp32)
            nc.scalar.activation(AJ[:, :], C3, AF.Copy)

            # ---- pair-distance branch: rinv = 1/|p_i - p_j| per pair ----
            D = sb.tile([3, 3], fp32)
            nc.vector.tensor_sub(D[:, :], PA, PB)
            DD = sb.tile([3, 3], fp32)
            nc.vector.tensor_mul(DD[:, :], D[:, :], D[:, :])
            R2p = ps.tile([1, 3], fp32)
            nc.tensor.matmul(R2p[:, :], ON3, DD[:, :], start=True, stop=True)
            # Quake rsqrt straight off PSUM: y0_bits = 0x5f3759df - (x>>1)
            #                              = ((x>>1) ^ -1) + 0x5f3759e0
            Y = sb.tile([1, 3], fp32)
            nc.vector.tensor_scalar(
                Y.bitcast(i32)[:, :], R2p.bitcast(i32)[:, :],
                1, -1, OP.arith_shift_right, OP.bitwise_xor,
            )
            nc.vector.tensor_scalar(
                Y.bitcast(i32)[:, :], Y.bitcast(i32)[:, :],
                0x5F3759E0, None, OP.add,
            )
            # Newton: y <- y * (1.5 - 0.5 x y^2); ~1.8e-3 rel after one step
            T = sb.tile([1, 3], fp32)
            nc.vector.tensor_mul(T[:, :], Y[:, :], Y[:, :])
            nc.vector.tensor_mul(T[:, :], T[:, :], R2p[:, :])
            nc.vector.tensor_scalar(T[:, :], T[:, :], -0.5, 1.5, OP.mult, OP.add)
            nc.vector.tensor_mul(Y[:, :], Y[:, :], T[:, :])

            # ---- fused dual MLP (hidden dims stacked: charge 0:32, sr 32:64) ----
            def silu(out_ap, in_psum, bias_ap, rows):
                if not silu_via_sigmoid:
                    nc.scalar.activation(out_ap, in_psum, AF.Silu, bias=bias_ap, scale=1.0)
                    return
                X1 = sb.tile([rows, 3], fp32, tag=f"x{rows}")
                nc.vector.tensor_scalar(X1[:, :], in_psum, bias_ap, None, OP.add)
                SG = sb.tile([rows, 3], fp32, tag=f"s{rows}")
                nc.scalar.activation(SG[:, :], in_psum, AF.Sigmoid, bias=bias_ap, scale=1.0)
                nc.vector.tensor_mul(out_ap, X1[:, :], SG[:, :])

            M1 = ps.tile([64, 3], fp32)
            nc.tensor.matmul(M1[:, :], W1, XT, start=True, stop=True)
            H1 = sb.tile([64, 3], bf16)
            silu(H1[:, :], M1[:, :], B1, 64)
            M2 = ps.tile([64, 3], fp32)
            nc.tensor.matmul(M2[:, :], W2, H1[:, :], start=True, stop=True)
            H2 = sb.tile([64, 3], bf16)
            silu(H2[:, :], M2[:, :], B2, 64)
            T3q = ps.tile([1, 3], fp32)  # per-atom raw_q (minus cb3, which cancels)
            nc.tensor.matmul(T3q[:, :], w3q, H2[0:32, :], start=True, stop=True)
            T3s = ps.tile([1, 3], fp32)  # per-atom short-range E (minus sb3)
            nc.tensor.matmul(T3s[:, :], w3s, H2[32:64, :], start=True, stop=True)

            # ---- charge correction + energy assembly ----
            # SR = sum_i T3s_i (3*sb3 is folded into the final add)
            SR = sb.tile([1, 1], fp32)
            nc.vector.reduce_sum(SR[:, :], T3s[:, :], axis=AX.X)
            # corr = -(sum T3q)/3 + C/3 in ONE fused op: out = T3q * -1/3,
            # accum = reduce_add(out) then + C/3 (scalar2 post-reduce)
            CR = sb.tile([1, 1], fp32)
            CAj = sb.tile([1, 3], fp32)
            nc.vector.tensor_scalar(
                CAj[:, :], T3q[:, :], -1.0 / 3.0, C3, OP.mult, OP.add,
                accum_out=CR[0:1, 0:1],
            )
            OUT = sb.tile([1, 4], fp32)
            nc.vector.tensor_scalar(
                OUT[0:1, 1:4], T3q[:, :], CR[0:1, 0:1], None, OP.add,
            )
            PQ = sb.tile([1, 3], fp32)
            nc.vector.tensor_scalar(
                PQ[0:1, 0:2], OUT[0:1, 2:4], OUT[0:1, 1:2], None, OP.mult,
            )
            nc.vector.tensor_mul(PQ[0:1, 2:3], OUT[0:1, 2:3], OUT[0:1, 3:4])
            EP3 = sb.tile([1, 3], fp32)
            nc.vector.tensor_mul(EP3[:, :], PQ[:, :], Y[:, :])
            EE = sb.tile([1, 1], fp32)
            nc.vector.reduce_sum(EE[:, :], EP3[:, :], axis=AX.X)
            nc.vector.tensor_scalar(
                OUT[0:1, 0:1], EE[:, :], SR[0:1, 0:1], SB3, OP.add, OP.add,
            )
            nc.sync.dma_start(out=od[:, :], in_=OUT[:, :])

    return nc


def _get_nc():
    global _NC
    if _NC is None:
        _NC = _build_nc()
    return _NC


def _pack_inputs(inputs):
    import ml_dtypes

    g = lambda k: np.asarray(inputs[k], dtype=np.float32)
    pack = np.zeros((64, _F), dtype=np.float32)
    pack[0:32, _C_B1] = g("cb1")
    pack[32:64, _C_B1] = g("sb1")
    pack[0:32, _C_B2] = g("cb2")
    pack[32:64, _C_B2] = g("sb2")
    pos = g("positions")
    pack[0:3, _C_PA:_C_PA + 3] = pos[list(_II)].T
    pack[0:3, _C_PB:_C_PB + 3] = pos[list(_JJ)].T
    pack[0:3, _C_ON] = 1.0
    pack[0, _C_C3] = float(np.asarray(inputs["charge_state"])) / 3.0
    pack[0, _C_SB3] = 3.0 * g("sb3")[0]

    packb = np.zeros((64, _FB), dtype=np.float32)
    packb[0:32, _B_W2:_B_W2 + 32] = g("cw2")
    packb[32:64, _B_W2 + 32:_B_W2 + 64] = g("sw2")
    packb[0:32, _B_W3] = g("cw3")[:, 0]
    packb[32:64, _B_W3] = g("sw3")[:, 0]
    packb[0:16, _B_W1:_B_W1 + 32] = g("cw1")
    packb[0:16, _B_W1 + 32:_B_W1 + 64] = g("sw1")
    packb[0:16, _B_XT:_B_XT + 3] = g("features").T
    pb = np.ascontiguousarray(packb.astype(ml_dtypes.bfloat16))
    pack[:, _C_BF:_C_BF + _FB // 2] = pb.view(np.uint32).view(np.float32)
    return pack


def _install_ntff_hook():
    # The image's antenv lacks axon_hooks; inject it and wire the ctypes
    # NTFF profile hook from trn_boot so trace=True yields exec_time_ns.
    try:
        from antenv.axon_hooks import get_axon_ntff_profile_hook  # noqa: F401
        return
    except ImportError:
        pass
    import types

    import antenv
    from trn_agent_boot.trn_boot import _ntff_profile_via_ctypes

    mod = types.ModuleType("antenv.axon_hooks")
    state = {"hook": None}
    mod.set_axon_ntff_profile_hook = lambda h: state.__setitem__("hook", h)
    mod.get_axon_ntff_profile_hook = lambda: state["hook"]
    sys.modules["antenv.axon_hooks"] = mod
    antenv.axon_hooks = mod
    mod.set_axon_ntff_profile_hook(
        _ntff_profile_via_ctypes("/opt/axon/libaxon_pjrt.so")
    )


def run(inputs, trace=False):
    """Run on hardware across 8 cores (replicated). Returns ((E, q), exec_ns)."""
    from concourse.bass_utils import run_bass_kernel_spmd

    if trace:
        _install_ntff_hook()
    nc = _get_nc()
    pack = _pack_inputs(inputs)
    in_maps = [{"pack": pack} for _ in range(8)]
    res = run_bass_kernel_spmd(nc, in_maps, list(range(8)), trace=trace)
    out = np.asarray(res.results[0]["out"], dtype=np.float32)
    E = np.asarray(out[0, 0], dtype=np.float32)
    q = out[0, 1:4].astype(np.float32)
    return (E, q), res.exec_time_ns


def kernel(**inputs):
    (E, q), _ = run(inputs)
    return (E, q)


# revision 45
# speedup vs baseline: 1.0677x; 1.0114x over previous
"""Trainium2 Bass kernel for Ag3LESModel (nn_Ag3LESModel_52158082842739).

Computes, for a 3-atom system:
  raw_q  = MLP_c(features)[:, 0]                  (16->32->32->1, SiLU)
  latent = raw_q + (charge_state - sum(raw_q))/3
  E_lr   = sum_{i<j} latent_i latent_j / |p_i - p_j|
  E_sr   = sum(MLP_s(features)[:, 0])             (16->32->32->1, SiLU)
  returns (E_lr + E_sr, latent)

Device strategy (single NeuronCore, replicated across the 8 cores):
  - Both MLPs fused into one stack: layer1/2 run as single matmuls over a
    64-wide concatenated hidden dim; layer 3 splits into two (1,3) matmuls.
  - cb3 cancels out of latent_q analytically, so it is dropped. sb3 is
    folded into the E_sr matmul via an extra ones row (lhsT = [sw3; sb3]).
  - 1/r via Quake rsqrt (int bit trick + 2 Newton steps) on the vector
    engine: avoids loading the ACT sqrt table set (only Silu's set loads).
  - Final energy assembled with one fused tensor_tensor_reduce.
All inputs are packed host-side into one (65,142) f32 tile -> single DMA in,
single (1,4) DMA out: [E, q0, q1, q2].
"""

import os
import sys

import numpy as np

if "/opt/trn_rl_repo" not in sys.path:
    sys.path.insert(0, "/opt/trn_rl_repo")

N = 3
_II = (0, 0, 1)
_JJ = (1, 2, 2)

# One f32 packed tile per core: cols 0:11 hold f32 data (biases, pair
# geometry, scalars); cols 11:77 hold a bf16 sub-pack (matmul weights +
# features) stored as raw 32-bit words and bitcast to bf16 on-chip.
_F = 77
_C_B1 = 0      # col 0, rows 0:64 : [cb1; sb1]
_C_B2 = 1      # col 1, rows 0:64 : [cb2; sb2]
_C_PA = 2      # cols 2:5, rows 0:3 : positions[II].T
_C_PB = 5      # cols 5:8, rows 0:3 : positions[JJ].T
_C_ON = 8      # col 8, rows 0:3 : ones
_C_C3 = 9      # col 9, row 0 : charge_state / 3
_C_SB3 = 10    # col 10, row 0 : 3 * sb3
_C_BF = 11     # cols 11:77 : bf16 sub-pack (132 bf16 columns)

# bf16 sub-pack column layout (in bf16 units, offset by 2*_C_BF on chip)
_FB = 132
_B_W2 = 0      # cols 0:64, rows 0:64 : blockdiag(cw2, sw2)
_B_W3 = 64     # col 64: rows 0:32 cw3, rows 32:64 sw3
_B_W1 = 65     # cols 65:129, rows 0:16 : [cw1 | sw1]
_B_XT = 129    # cols 129:132, rows 0:16 : features.T

_NC = None
_DRAIN_PATCHED = False


def _patch_drain_wait_split(tile, mybir, max_waits=1):
    # Replace Tile's kernel tail. Stock tail = drain + all-engine barrier +
    # semaphore clear + second all-engine barrier: the two EVSEM butterflies
    # cost ~7us on silicon. Here: a chain of 1-wait drains on the sync engine
    # (the drain encoding holds only one wait; the stock 5-wait drain fails
    # codegen), the last of which bumps a fresh 'done' semaphore; gpsimd
    # waits on it, then resets DMA queues and clears all semaphores
    # (including 'done'), so the NEFF stays re-executable. Once the drain
    # chain has retired, every engine has passed its last semaphore wait,
    # so the barrier-free clear cannot strand a waiter.
    global _DRAIN_PATCHED
    if _DRAIN_PATCHED:
        return
    _DRAIN_PATCHED = True
    orig = tile.TileContext._drain_and_barrier

    def patched(self, tick_clock, wait_clock):
        from concourse.vector_clock import ScopedClock

        nc = self.nc
        drain_inst = nc.sync.drain()
        wait_clock.add_sem_waits(
            drain_inst.ins, ScopedClock({None: tick_clock.global_clock})
        )
        si = drain_inst.ins.sync_info
        waits = list(si.on_wait) if si is not None else []
        upds = list(si.on_update) if si is not None else []
        chain = [drain_inst]
        if len(waits) > max_waits:
            drain_inst.ins.sync_info = mybir.SyncInfo(
                on_wait=waits[:max_waits], on_update=upds
            )
            rest = waits[max_waits:]
            while rest:
                extra = nc.sync.drain()
                extra.ins.sync_info = mybir.SyncInfo(
                    on_wait=rest[:max_waits], on_update=[]
                )
                chain.append(extra)
                rest = rest[max_waits:]
        done = nc.alloc_semaphore("tail_done")
        chain[-1].then_inc(done, 1)

        assert self.sems is not None
        popped = nc._tile_sem_poison_stack.pop()
        assert popped is self._sem_poison
        nc.gpsimd.wait_ge(done, 1)
        nc.clear_and_free_semaphores(
            list(self.sems.allocated().values()) + [done]
        )

    patched._orig = orig
    tile.TileContext._drain_and_barrier = patched


def _build_nc(silu_via_sigmoid=False):
    # silu_via_sigmoid: CoreSim has no Silu table; build silu(x) = x*sigmoid(x)
    # out of Sigmoid + vector ops for the sim gate. HW uses Silu directly.
    import concourse.bass as bass
    import concourse.mybir as mybir
    import concourse.tile as tile

    _patch_drain_wait_split(tile, mybir)

    fp32 = mybir.dt.float32
    bf16 = mybir.dt.bfloat16
    i32 = mybir.dt.int32
    AF = mybir.ActivationFunctionType
    OP = mybir.AluOpType
    AX = mybir.AxisListType

    nc = bass.Bass("TRN2", target_bir_lowering=False, debug=False)
    # Drop the const-pool memsets Bass.__init__ emits (0.0 / 1.0 / bf16-1.0 /
    # u8-127): nothing in this kernel reads them, and they burn ~400ns of
    # gpsimd time at the head of the measured window.
    _insts = nc.m.functions[0].blocks[0].instructions
    for _i in range(len(_insts) - 1, -1, -1):
        if type(_insts[_i]).__name__ == "InstMemset":
            del _insts[_i]
    pk = nc.declare_dram_parameter("pack", [64, _F], fp32, isOutput=False)
    od = nc.declare_dram_parameter("out", [1, 4], fp32, isOutput=True)

    with tile.TileContext(nc) as tc:
        with (
            tc.tile_pool(name="sb", bufs=1) as sb,
            tc.tile_pool(name="ps", bufs=1, space="PSUM") as ps,
        ):
            P = sb.tile([64, _F], fp32)
            nc.sync.dma_start(out=P[:, :], in_=pk[:, :])

            PV = P.bitcast(bf16)  # (64, 2*_F) bf16 view of the same bytes
            _o = 2 * _C_BF
            W2 = PV[0:64, _o + _B_W2:_o + _B_W2 + 64]
            w3q = PV[0:32, _o + _B_W3:_o + _B_W3 + 1]
            w3s = PV[32:64, _o + _B_W3:_o + _B_W3 + 1]
            W1 = PV[0:16, _o + _B_W1:_o + _B_W1 + 64]
            XT = PV[0:16, _o + _B_XT:_o + _B_XT + 3]
            B1 = P[0:64, _C_B1:_C_B1 + 1]
            B2 = P[0:64, _C_B2:_C_B2 + 1]
            PA = P[0:3, _C_PA:_C_PA + 3]
            PB = P[0:3, _C_PB:_C_PB + 3]
            ON3 = P[0:3, _C_ON:_C_ON + 1]
            C3 = P[0:1, _C_C3:_C_C3 + 1]
            SB3 = P[0:1, _C_SB3:_C_SB3 + 1]

            # ACT primer: the ACTIVATE encoding supports a single sync wait,
            # but silu1 would need two (PE matmul + DMA'd bias tile). This
            # op makes ACT observe the input-DMA semaphore first, so silu1
            # only waits on PE.
            AJ = sb.tile([1, 1], fp32)
            nc.scalar.activation(AJ[:, :], C3, AF.Copy)

            # ---- pair-distance branch: rinv = 1/|p_i - p_j| per pair ----
            D = sb.tile([3, 3], fp32)
            nc.vector.tensor_sub(D[:, :], PA, PB)
            DD = sb.tile([3, 3], fp32)
            nc.gpsimd.tensor_mul(DD[:, :], D[:, :], D[:, :])
            R2p = ps.tile([1, 3], fp32)
            nc.tensor.matmul(R2p[:, :], ON3, DD[:, :], start=True, stop=True)
            # Quake rsqrt straight off PSUM: y0_bits = 0x5f3759df - (x>>1)
            #                              = ((x>>1) ^ -1) + 0x5f3759e0
            Y = sb.tile([1, 3], fp32)
            nc.vector.tensor_scalar(
                Y.bitcast(i32)[:, :], R2p.bitcast(i32)[:, :],
                1, -1, OP.arith_shift_right, OP.bitwise_xor,
            )
            nc.vector.tensor_scalar(
                Y.bitcast(i32)[:, :], Y.bitcast(i32)[:, :],
                0x5F3759E0, None, OP.add,
            )
            # Newton: y <- y * (1.5 - 0.5 x y^2); ~1.8e-3 rel after one step
            T = sb.tile([1, 3], fp32)
            nc.vector.tensor_mul(T[:, :], Y[:, :], Y[:, :])
            nc.vector.tensor_mul(T[:, :], T[:, :], R2p[:, :])
            nc.vector.tensor_scalar(T[:, :], T[:, :], -0.5, 1.5, OP.mult, OP.add)
            nc.vector.tensor_mul(Y[:, :], Y[:, :], T[:, :])

            # ---- fused dual MLP (hidden dims stacked: charge 0:32, sr 32:64) ----
            def silu(out_ap, in_psum, bias_ap, rows):
                if not silu_via_sigmoid:
                    nc.scalar.activation(out_ap, in_psum, AF.Silu, bias=bias_ap, scale=1.0)
                    return
                X1 = sb.tile([rows, 3], fp32, tag=f"x{rows}")
                nc.vector.tensor_scalar(X1[:, :], in_psum, bias_ap, None, OP.add)
                SG = sb.tile([rows, 3], fp32, tag=f"s{rows}")
                nc.scalar.activation(SG[:, :], in_psum, AF.Sigmoid, bias=bias_ap, scale=1.0)
                nc.vector.tensor_mul(out_ap, X1[:, :], SG[:, :])

            M1 = ps.tile([64, 3], fp32)
            nc.tensor.matmul(M1[:, :], W1, XT, start=True, stop=True)
            H1 = sb.tile([64, 3], bf16)
            silu(H1[:, :], M1[:, :], B1, 64)
            M2 = ps.tile([64, 3], fp32)
            nc.tensor.matmul(M2[:, :], W2, H1[:, :], start=True, stop=True)
            H2 = sb.tile([64, 3], bf16)
            silu(H2[:, :], M2[:, :], B2, 64)
            T3q = ps.tile([1, 3], fp32)  # per-atom raw_q (minus cb3, which cancels)
            nc.tensor.matmul(T3q[:, :], w3q, H2[0:32, :], start=True, stop=True)
            T3s = ps.tile([1, 3], fp32)  # per-atom short-range E (minus sb3)
            nc.tensor.matmul(T3s[:, :], w3s, H2[32:64, :], start=True, stop=True)

            # ---- charge correction + energy assembly ----
            # SR = sum_i T3s_i (3*sb3 is folded into the final add)
            SR = sb.tile([1, 1], fp32)
            nc.vector.reduce_sum(SR[:, :], T3s[:, :], axis=AX.X)
            # corr = -(sum T3q)/3 + C/3 in ONE fused op: out = T3q * -1/3,
            # accum = reduce_add(out) then + C/3 (scalar2 post-reduce)
            CR = sb.tile([1, 1], fp32)
            CAj = sb.tile([1, 3], fp32)
            nc.vector.tensor_scalar(
                CAj[:, :], T3q[:, :], -1.0 / 3.0, C3, OP.mult, OP.add,
                accum_out=CR[0:1, 0:1],
            )
            OUT = sb.tile([1, 4], fp32)
            nc.vector.tensor_scalar(
                OUT[0:1, 1:4], T3q[:, :], CR[0:1, 0:1], None, OP.add,
            )
            PQ = sb.tile([1, 3], fp32)
            nc.vector.tensor_mul(
                PQ[0:1, 0:2], OUT[0:1, 2:4],
                OUT[0:1, 1:2].broadcast_to((1, 2)),
            )
            nc.vector.tensor_mul(PQ[0:1, 2:3], OUT[0:1, 2:3], OUT[0:1, 3:4])
            # EE = sum(PQ * rinv), fused elementwise-mul + accumulate
            EPj = sb.tile([1, 3], fp32)
            EE = sb.tile([1, 1], fp32)
            nc.vector.scalar_tensor_tensor(
                EPj[:, :], PQ[:, :], 1.0, Y[:, :], OP.mult, OP.mult,
                accum_out=EE[0:1, 0:1],
            )
            nc.vector.tensor_scalar(
                OUT[0:1, 0:1], EE[:, :], SR[0:1, 0:1], SB3, OP.add, OP.add,
            )
            nc.sync.dma_start(out=od[:, :], in_=OUT[:, :])

    return nc


def _get_nc():
    global _NC
    if _NC is None:
        _NC = _build_nc()
    return _NC


def _pack_inputs(inputs):
    import ml_dtypes

    g = lambda k: np.asarray(inputs[k], dtype=np.float32)
    pack = np.zeros((64, _F), dtype=np.float32)
    pack[0:32, _C_B1] = g("cb1")
    pack[32:64, _C_B1] = g("sb1")
    pack[0:32, _C_B2] = g("cb2")
    pack[32:64, _C_B2] = g("sb2")
    pos = g("positions")
    pack[0:3, _C_PA:_C_PA + 3] = pos[list(_II)].T
    pack[0:3, _C_PB:_C_PB + 3] = pos[list(_JJ)].T
    pack[0:3, _C_ON] = 1.0
    pack[0, _C_C3] = float(np.asarray(inputs["charge_state"])) / 3.0
    pack[0, _C_SB3] = 3.0 * g("sb3")[0]

    packb = np.zeros((64, _FB), dtype=np.float32)
    packb[0:32, _B_W2:_B_W2 + 32] = g("cw2")
    packb[32:64, _B_W2 + 32:_B_W2 + 64] = g("sw2")
    packb[0:32, _B_W3] = g("cw3")[:, 0]
    packb[32:64, _B_W3] = g("sw3")[:, 0]
    packb[0:16, _B_W1:_B_W1 + 32] = g("cw1")
    packb[0:16, _B_W1 + 32:_B_W1 + 64] = g("sw1")
    packb[0:16, _B_XT:_B_XT + 3] = g("features").T
    pb = np.ascontiguousarray(packb.astype(ml_dtypes.bfloat16))
    pack[:, _C_BF:_C_BF + _FB // 2] = pb.view(np.uint32).view(np.float32)
    return pack


def _install_ntff_hook():
    # The image's antenv lacks axon_hooks; inject it and wire the ctypes
    # NTFF profile hook from trn_boot so trace=True yields exec_time_ns.
    try:
        from antenv.axon_hooks import get_axon_ntff_profile_hook  # noqa: F401
        return
    except ImportError:
        pass
    import types

    import antenv
    from trn_agent_boot.trn_boot import _ntff_profile_via_ctypes

    mod = types.ModuleType("antenv.axon_hooks")
    state = {"hook": None}
    mod.set_axon_ntff_profile_hook = lambda h: state.__setitem__("hook", h)
    mod.get_axon_ntff_profile_hook = lambda: state["hook"]
    sys.modules["antenv.axon_hooks"] = mod
    antenv.axon_hooks = mod
    mod.set_axon_ntff_profile_hook(
        _ntff_profile_via_ctypes("/opt/axon/libaxon_pjrt.so")
    )


def run(inputs, trace=False):
    """Run on hardware across 8 cores (replicated). Returns ((E, q), exec_ns)."""
    from concourse.bass_utils import run_bass_kernel_spmd

    if trace:
        _install_ntff_hook()
    nc = _get_nc()
    pack = _pack_inputs(inputs)
    in_maps = [{"pack": pack} for _ in range(8)]
    res = run_bass_kernel_spmd(nc, in_maps, list(range(8)), trace=trace)
    out = np.asarray(res.results[0]["out"], dtype=np.float32)
    E = np.asarray(out[0, 0], dtype=np.float32)
    q = out[0, 1:4].astype(np.float32)
    return (E, q), res.exec_time_ns


def kernel(**inputs):
    (E, q), _ = run(inputs)
    return (E, q)


# revision 47
# speedup vs baseline: 1.0764x; 1.0082x over previous
"""Trainium2 Bass kernel for Ag3LESModel (nn_Ag3LESModel_52158082842739).

Computes, for a 3-atom system:
  raw_q  = MLP_c(features)[:, 0]                  (16->32->32->1, SiLU)
  latent = raw_q + (charge_state - sum(raw_q))/3
  E_lr   = sum_{i<j} latent_i latent_j / |p_i - p_j|
  E_sr   = sum(MLP_s(features)[:, 0])             (16->32->32->1, SiLU)
  returns (E_lr + E_sr, latent)

Device strategy (single NeuronCore, replicated across the 8 cores):
  - Both MLPs fused into one stack: layer1/2 run as single matmuls over a
    64-wide concatenated hidden dim; layer 3 splits into two (1,3) matmuls.
  - cb3 cancels out of latent_q analytically, so it is dropped. sb3 is
    folded into the E_sr matmul via an extra ones row (lhsT = [sw3; sb3]).
  - 1/r via Quake rsqrt (int bit trick + 2 Newton steps) on the vector
    engine: avoids loading the ACT sqrt table set (only Silu's set loads).
  - Final energy assembled with one fused tensor_tensor_reduce.
All inputs are packed host-side into one (65,142) f32 tile -> single DMA in,
single (1,4) DMA out: [E, q0, q1, q2].
"""

import os
import sys

import numpy as np

if "/opt/trn_rl_repo" not in sys.path:
    sys.path.insert(0, "/opt/trn_rl_repo")

N = 3
_II = (0, 0, 1)
_JJ = (1, 2, 2)

# One f32 packed tile per core: cols 0:11 hold f32 data (biases, pair
# geometry, scalars); cols 11:77 hold a bf16 sub-pack (matmul weights +
# features) stored as raw 32-bit words and bitcast to bf16 on-chip.
_F = 77
_C_B1 = 0      # col 0, rows 0:64 : [cb1; sb1]
_C_B2 = 1      # col 1, rows 0:64 : [cb2; sb2]
_C_PA = 2      # cols 2:5, rows 0:3 : positions[II].T
_C_PB = 5      # cols 5:8, rows 0:3 : positions[JJ].T
_C_ON = 8      # col 8, rows 0:3 : ones
_C_C3 = 9      # col 9, row 0 : charge_state / 3
_C_SB3 = 10    # col 10, row 0 : 3 * sb3
_C_BF = 11     # cols 11:77 : bf16 sub-pack (132 bf16 columns)

# bf16 sub-pack column layout (in bf16 units, offset by 2*_C_BF on chip)
_FB = 132
_B_W2 = 0      # cols 0:64, rows 0:64 : blockdiag(cw2, sw2)
_B_W3 = 64     # col 64: rows 0:32 cw3, rows 32:64 sw3
_B_W1 = 65     # cols 65:129, rows 0:16 : [cw1 | sw1]
_B_XT = 129    # cols 129:132, rows 0:16 : features.T

_NC = None
_DRAIN_PATCHED = False


def _patch_drain_wait_split(tile, mybir, max_waits=1):
    # Replace Tile's kernel tail. Stock tail = drain + all-engine barrier +
    # semaphore clear + second all-engine barrier: the two EVSEM butterflies
    # cost ~7us on silicon. Here: a chain of 1-wait drains on the sync engine
    # (the drain encoding holds only one wait; the stock 5-wait drain fails
    # codegen), the last of which bumps a fresh 'done' semaphore; gpsimd
    # waits on it, then resets DMA queues and clears all semaphores
    # (including 'done'), so the NEFF stays re-executable. Once the drain
    # chain has retired, every engine has passed its last semaphore wait,
    # so the barrier-free clear cannot strand a waiter.
    global _DRAIN_PATCHED
    if _DRAIN_PATCHED:
        return
    _DRAIN_PATCHED = True
    orig = tile.TileContext._drain_and_barrier

    def patched(self, tick_clock, wait_clock):
        from concourse.vector_clock import ScopedClock

        nc = self.nc
        drain_inst = nc.sync.drain()
        wait_clock.add_sem_waits(
            drain_inst.ins, ScopedClock({None: tick_clock.global_clock})
        )
        si = drain_inst.ins.sync_info
        waits = list(si.on_wait) if si is not None else []
        upds = list(si.on_update) if si is not None else []
        chain = [drain_inst]
        if len(waits) > max_waits:
            drain_inst.ins.sync_info = mybir.SyncInfo(
                on_wait=waits[:max_waits], on_update=upds
            )
            rest = waits[max_waits:]
            while rest:
                extra = nc.sync.drain()
                extra.ins.sync_info = mybir.SyncInfo(
                    on_wait=rest[:max_waits], on_update=[]
                )
                chain.append(extra)
                rest = rest[max_waits:]
        done = nc.alloc_semaphore("tail_done")
        chain[-1].then_inc(done, 1)

        assert self.sems is not None
        popped = nc._tile_sem_poison_stack.pop()
        assert popped is self._sem_poison
        nc.gpsimd.wait_ge(done, 1)
        nc.clear_and_free_semaphores(
            list(self.sems.allocated().values()) + [done]
        )

    patched._orig = orig
    tile.TileContext._drain_and_barrier = patched


def _build_nc(silu_via_sigmoid=False):
    # silu_via_sigmoid: CoreSim has no Silu table; build silu(x) = x*sigmoid(x)
    # out of Sigmoid + vector ops for the sim gate. HW uses Silu directly.
    import concourse.bass as bass
    import concourse.mybir as mybir
    import concourse.tile as tile

    _patch_drain_wait_split(tile, mybir)

    fp32 = mybir.dt.float32
    bf16 = mybir.dt.bfloat16
    i32 = mybir.dt.int32
    AF = mybir.ActivationFunctionType
    OP = mybir.AluOpType
    AX = mybir.AxisListType

    nc = bass.Bass("TRN2", target_bir_lowering=False, debug=False)
    # Drop the const-pool memsets Bass.__init__ emits (0.0 / 1.0 / bf16-1.0 /
    # u8-127): nothing in this kernel reads them, and they burn ~400ns of
    # gpsimd time at the head of the measured window.
    _insts = nc.m.functions[0].blocks[0].instructions
    for _i in range(len(_insts) - 1, -1, -1):
        if type(_insts[_i]).__name__ == "InstMemset":
            del _insts[_i]
    pk = nc.declare_dram_parameter("pack", [64, _F], fp32, isOutput=False)
    od = nc.declare_dram_parameter("out", [1, 4], fp32, isOutput=True)

    with tile.TileContext(nc) as tc:
        with (
            tc.tile_pool(name="sb", bufs=1) as sb,
            tc.tile_pool(name="ps", bufs=1, space="PSUM") as ps,
        ):
            P = sb.tile([64, _F], fp32)
            nc.sync.dma_start(out=P[:, :], in_=pk[:, :])

            PV = P.bitcast(bf16)  # (64, 2*_F) bf16 view of the same bytes
            _o = 2 * _C_BF
            W2 = PV[0:64, _o + _B_W2:_o + _B_W2 + 64]
            w3q = PV[0:32, _o + _B_W3:_o + _B_W3 + 1]
            w3s = PV[32:64, _o + _B_W3:_o + _B_W3 + 1]
            W1 = PV[0:16, _o + _B_W1:_o + _B_W1 + 64]
            XT = PV[0:16, _o + _B_XT:_o + _B_XT + 3]
            B1 = P[0:64, _C_B1:_C_B1 + 1]
            B2 = P[0:64, _C_B2:_C_B2 + 1]
            PA = P[0:3, _C_PA:_C_PA + 3]
            PB = P[0:3, _C_PB:_C_PB + 3]
            ON3 = P[0:3, _C_ON:_C_ON + 1]
            C3 = P[0:1, _C_C3:_C_C3 + 1]
            SB3 = P[0:1, _C_SB3:_C_SB3 + 1]

            # ACT primer: the ACTIVATE encoding supports a single sync wait,
            # but silu1 would need two (PE matmul + DMA'd bias tile). This
            # op makes ACT observe the input-DMA semaphore first, so silu1
            # only waits on PE.
            AJ = sb.tile([1, 1], fp32)
            nc.scalar.activation(AJ[:, :], C3, AF.Copy)

            # ---- pair-distance branch: rinv = 1/|p_i - p_j| per pair ----
            D = sb.tile([3, 3], fp32)
            nc.vector.tensor_sub(D[:, :], PA, PB)
            DD = sb.tile([3, 3], fp32)
            nc.gpsimd.tensor_mul(DD[:, :], D[:, :], D[:, :])
            R2p = ps.tile([1, 3], fp32)
            nc.tensor.matmul(R2p[:, :], ON3, DD[:, :], start=True, stop=True)
            # Quake rsqrt straight off PSUM: y0_bits = 0x5f3759df - (x>>1)
            #                              = ((x>>1) ^ -1) + 0x5f3759e0
            Y = sb.tile([1, 3], fp32)
            nc.vector.tensor_scalar(
                Y.bitcast(i32)[:, :], R2p.bitcast(i32)[:, :],
                1, -1, OP.arith_shift_right, OP.bitwise_xor,
            )
            nc.vector.tensor_scalar(
                Y.bitcast(i32)[:, :], Y.bitcast(i32)[:, :],
                0x5F3759E0, None, OP.add,
            )
            # Newton: y <- y * (1.5 - 0.5 x y^2); ~1.8e-3 rel after one step
            T = sb.tile([1, 3], fp32)
            nc.vector.tensor_mul(T[:, :], Y[:, :], Y[:, :])
            nc.vector.tensor_mul(T[:, :], T[:, :], R2p[:, :])
            nc.vector.tensor_scalar(T[:, :], T[:, :], -0.5, 1.5, OP.mult, OP.add)
            nc.vector.tensor_mul(Y[:, :], Y[:, :], T[:, :])

            # ---- fused dual MLP (hidden dims stacked: charge 0:32, sr 32:64) ----
            def silu(out_ap, in_psum, bias_ap, rows):
                if not silu_via_sigmoid:
                    nc.scalar.activation(out_ap, in_psum, AF.Silu, bias=bias_ap, scale=1.0)
                    return
                X1 = sb.tile([rows, 3], fp32, tag=f"x{rows}")
                nc.vector.tensor_scalar(X1[:, :], in_psum, bias_ap, None, OP.add)
                SG = sb.tile([rows, 3], fp32, tag=f"s{rows}")
                nc.scalar.activation(SG[:, :], in_psum, AF.Sigmoid, bias=bias_ap, scale=1.0)
                nc.vector.tensor_mul(out_ap, X1[:, :], SG[:, :])

            M1 = ps.tile([64, 3], fp32)
            nc.tensor.matmul(M1[:, :], W1, XT, start=True, stop=True)
            H1 = sb.tile([64, 3], bf16)
            silu(H1[:, :], M1[:, :], B1, 64)
            M2 = ps.tile([64, 3], fp32)
            nc.tensor.matmul(M2[:, :], W2, H1[:, :], start=True, stop=True)
            H2 = sb.tile([64, 3], bf16)
            silu(H2[:, :], M2[:, :], B2, 64)
            T3q = ps.tile([1, 3], fp32)  # per-atom raw_q (minus cb3, which cancels)
            nc.tensor.matmul(T3q[:, :], w3q, H2[0:32, :], start=True, stop=True)
            T3s = ps.tile([1, 3], fp32)  # per-atom short-range E (minus sb3)
            nc.tensor.matmul(T3s[:, :], w3s, H2[32:64, :], start=True, stop=True)

            # ---- charge correction + energy assembly ----
            # SR = sum_i T3s_i (3*sb3 is folded into the final add)
            SR = sb.tile([1, 1], fp32)
            nc.vector.reduce_sum(SR[:, :], T3s[:, :], axis=AX.X)
            # corr = -(sum T3q)/3 + C/3 in ONE fused op: out = T3q * -1/3,
            # accum = reduce_add(out) then + C/3 (scalar2 post-reduce)
            CR = sb.tile([1, 1], fp32)
            CAj = sb.tile([1, 3], fp32)
            nc.vector.tensor_scalar(
                CAj[:, :], T3q[:, :], -1.0 / 3.0, C3, OP.mult, OP.add,
                accum_out=CR[0:1, 0:1],
            )
            OUT = sb.tile([1, 4], fp32)
            nc.vector.scalar_tensor_tensor(
                OUT[0:1, 1:4], T3q[:, :], 1.0,
                CR[0:1, 0:1].broadcast_to((1, 3)), OP.mult, OP.add,
            )
            PQ = sb.tile([1, 3], fp32)
            nc.vector.tensor_mul(
                PQ[0:1, 0:2], OUT[0:1, 2:4],
                OUT[0:1, 1:2].broadcast_to((1, 2)),
            )
            nc.vector.tensor_mul(PQ[0:1, 2:3], OUT[0:1, 2:3], OUT[0:1, 3:4])
            # EE = sum(PQ * rinv), fused elementwise-mul + accumulate
            EPj = sb.tile([1, 3], fp32)
            EE = sb.tile([1, 1], fp32)
            nc.vector.scalar_tensor_tensor(
                EPj[:, :], PQ[:, :], 1.0, Y[:, :], OP.mult, OP.mult,
                accum_out=EE[0:1, 0:1],
            )
            nc.vector.scalar_tensor_tensor(
                OUT[0:1, 0:1], EE[:, :], SR[0:1, 0:1], SB3, OP.add, OP.add,
            )
            nc.sync.dma_start(out=od[:, :], in_=OUT[:, :])

    return nc


def _get_nc():
    global _NC
    if _NC is None:
        _NC = _build_nc()
    return _NC


def _pack_inputs(inputs):
    import ml_dtypes

    g = lambda k: np.asarray(inputs[k], dtype=np.float32)
    pack = np.zeros((64, _F), dtype=np.float32)
    pack[0:32, _C_B1] = g("cb1")
    pack[32:64, _C_B1] = g("sb1")
    pack[0:32, _C_B2] = g("cb2")
    pack[32:64, _C_B2] = g("sb2")
    pos = g("positions")
    pack[0:3, _C_PA:_C_PA + 3] = pos[list(_II)].T
    pack[0:3, _C_PB:_C_PB + 3] = pos[list(_JJ)].T
    pack[0:3, _C_ON] = 1.0
    pack[0, _C_C3] = float(np.asarray(inputs["charge_state"])) / 3.0
    pack[0, _C_SB3] = 3.0 * g("sb3")[0]

    packb = np.zeros((64, _FB), dtype=np.float32)
    packb[0:32, _B_W2:_B_W2 + 32] = g("cw2")
    packb[32:64, _B_W2 + 32:_B_W2 + 64] = g("sw2")
    packb[0:32, _B_W3] = g("cw3")[:, 0]
    packb[32:64, _B_W3] = g("sw3")[:, 0]
    packb[0:16, _B_W1:_B_W1 + 32] = g("cw1")
    packb[0:16, _B_W1 + 32:_B_W1 + 64] = g("sw1")
    packb[0:16, _B_XT:_B_XT + 3] = g("features").T
    pb = np.ascontiguousarray(packb.astype(ml_dtypes.bfloat16))
    pack[:, _C_BF:_C_BF + _FB // 2] = pb.view(np.uint32).view(np.float32)
    return pack


def _install_ntff_hook():
    # The image's antenv lacks axon_hooks; inject it and wire the ctypes
    # NTFF profile hook from trn_boot so trace=True yields exec_time_ns.
    try:
        from antenv.axon_hooks import get_axon_ntff_profile_hook  # noqa: F401
        return
    except ImportError:
        pass
    import types

    import antenv
    from trn_agent_boot.trn_boot import _ntff_profile_via_ctypes

    mod = types.ModuleType("antenv.axon_hooks")
    state = {"hook": None}
    mod.set_axon_ntff_profile_hook = lambda h: state.__setitem__("hook", h)
    mod.get_axon_ntff_profile_hook = lambda: state["hook"]
    sys.modules["antenv.axon_hooks"] = mod
    antenv.axon_hooks = mod
    mod.set_axon_ntff_profile_hook(
        _ntff_profile_via_ctypes("/opt/axon/libaxon_pjrt.so")
    )


def run(inputs, trace=False):
    """Run on hardware across 8 cores (replicated). Returns ((E, q), exec_ns)."""
    from concourse.bass_utils import run_bass_kernel_spmd

    if trace:
        _install_ntff_hook()
    nc = _get_nc()
    pack = _pack_inputs(inputs)
    in_maps = [{"pack": pack} for _ in range(8)]
    res = run_bass_kernel_spmd(nc, in_maps, list(range(8)), trace=trace)
    out = np.asarray(res.results[0]["out"], dtype=np.float32)
    E = np.asarray(out[0, 0], dtype=np.float32)
    q = out[0, 1:4].astype(np.float32)
    return (E, q), res.exec_time_ns


def kernel(**inputs):
    (E, q), _ = run(inputs)
    return (E, q)


# revision 50
# speedup vs baseline: 1.0769x; 1.0004x over previous
"""Trainium2 Bass kernel for Ag3LESModel (nn_Ag3LESModel_52158082842739).

Computes, for a 3-atom system:
  raw_q  = MLP_c(features)[:, 0]                  (16->32->32->1, SiLU)
  latent = raw_q + (charge_state - sum(raw_q))/3
  E_lr   = sum_{i<j} latent_i latent_j / |p_i - p_j|
  E_sr   = sum(MLP_s(features)[:, 0])             (16->32->32->1, SiLU)
  returns (E_lr + E_sr, latent)

Device strategy (single NeuronCore, replicated across the 8 cores):
  - Both MLPs fused into one stack: layer1/2 run as single matmuls over a
    64-wide concatenated hidden dim; layer 3 splits into two (1,3) matmuls.
  - cb3 cancels out of latent_q analytically, so it is dropped. sb3 is
    folded into the E_sr matmul via an extra ones row (lhsT = [sw3; sb3]).
  - 1/r via Quake rsqrt (int bit trick + 2 Newton steps) on the vector
    engine: avoids loading the ACT sqrt table set (only Silu's set loads).
  - Final energy assembled with one fused tensor_tensor_reduce.
All inputs are packed host-side into one (65,142) f32 tile -> single DMA in,
single (1,4) DMA out: [E, q0, q1, q2].
"""

import os
import sys

import numpy as np

if "/opt/trn_rl_repo" not in sys.path:
    sys.path.insert(0, "/opt/trn_rl_repo")

N = 3
_II = (0, 0, 1)
_JJ = (1, 2, 2)

# One f32 packed tile per core: cols 0:11 hold f32 data (biases, pair
# geometry, scalars); cols 11:77 hold a bf16 sub-pack (matmul weights +
# features) stored as raw 32-bit words and bitcast to bf16 on-chip.
_F = 77
_C_B1 = 0      # col 0, rows 0:64 : [cb1; sb1]
_C_B2 = 1      # col 1, rows 0:64 : [cb2; sb2]
_C_PA = 2      # cols 2:5, rows 0:3 : positions[II].T
_C_PB = 5      # cols 5:8, rows 0:3 : positions[JJ].T
_C_ON = 8      # col 8, rows 0:3 : ones
_C_C3 = 9      # col 9, row 0 : charge_state / 3
_C_SB3 = 10    # col 10, row 0 : 3 * sb3
_C_BF = 11     # cols 11:77 : bf16 sub-pack (132 bf16 columns)

# bf16 sub-pack column layout (in bf16 units, offset by 2*_C_BF on chip)
_FB = 132
_B_W2 = 0      # cols 0:64, rows 0:64 : blockdiag(cw2, sw2)
_B_W3 = 64     # col 64: rows 0:32 cw3, rows 32:64 sw3
_B_W1 = 65     # cols 65:129, rows 0:16 : [cw1 | sw1]
_B_XT = 129    # cols 129:132, rows 0:16 : features.T

_NC = None
_DRAIN_PATCHED = False


def _patch_drain_wait_split(tile, mybir, max_waits=1):
    # Replace Tile's kernel tail. Stock tail = drain + all-engine barrier +
    # semaphore clear + second all-engine barrier: the two EVSEM butterflies
    # cost ~7us on silicon. Here: a chain of 1-wait drains on the sync engine
    # (the drain encoding holds only one wait; the stock 5-wait drain fails
    # codegen), the last of which bumps a fresh 'done' semaphore; gpsimd
    # waits on it, then resets DMA queues and clears all semaphores
    # (including 'done'), so the NEFF stays re-executable. Once the drain
    # chain has retired, every engine has passed its last semaphore wait,
    # so the barrier-free clear cannot strand a waiter.
    global _DRAIN_PATCHED
    if _DRAIN_PATCHED:
        return
    _DRAIN_PATCHED = True
    orig = tile.TileContext._drain_and_barrier

    def patched(self, tick_clock, wait_clock):
        from concourse.vector_clock import ScopedClock

        nc = self.nc
        drain_inst = nc.sync.drain()
        wait_clock.add_sem_waits(
            drain_inst.ins, ScopedClock({None: tick_clock.global_clock})
        )
        si = drain_inst.ins.sync_info
        waits = list(si.on_wait) if si is not None else []
        # DMA completion semaphores arrive last on silicon (HBM write
        # receipt ~1us); wait on them at the END of the drain chain so the
        # cheap engine waits retire first.
        waits.sort(key=lambda w: ((w.ant_name or "").startswith("DMAHW"), w.ant_name))
        upds = list(si.on_update) if si is not None else []
        chain = [drain_inst]
        if len(waits) > max_waits:
            drain_inst.ins.sync_info = mybir.SyncInfo(
                on_wait=waits[:max_waits], on_update=upds
            )
            rest = waits[max_waits:]
            while rest:
                extra = nc.sync.drain()
                extra.ins.sync_info = mybir.SyncInfo(
                    on_wait=rest[:max_waits], on_update=[]
                )
                chain.append(extra)
                rest = rest[max_waits:]
        done = nc.alloc_semaphore("tail_done")
        chain[-1].then_inc(done, 1)

        assert self.sems is not None
        popped = nc._tile_sem_poison_stack.pop()
        assert popped is self._sem_poison
        nc.gpsimd.wait_ge(done, 1)
        nc.clear_and_free_semaphores(
            list(self.sems.allocated().values()) + [done]
        )

    patched._orig = orig
    tile.TileContext._drain_and_barrier = patched


def _build_nc(silu_via_sigmoid=False):
    # silu_via_sigmoid: CoreSim has no Silu table; build silu(x) = x*sigmoid(x)
    # out of Sigmoid + vector ops for the sim gate. HW uses Silu directly.
    import concourse.bass as bass
    import concourse.mybir as mybir
    import concourse.tile as tile

    _patch_drain_wait_split(tile, mybir)

    fp32 = mybir.dt.float32
    bf16 = mybir.dt.bfloat16
    i32 = mybir.dt.int32
    AF = mybir.ActivationFunctionType
    OP = mybir.AluOpType
    AX = mybir.AxisListType

    nc = bass.Bass("TRN2", target_bir_lowering=False, debug=False)
    # Drop the const-pool memsets Bass.__init__ emits (0.0 / 1.0 / bf16-1.0 /
    # u8-127): nothing in this kernel reads them, and they burn ~400ns of
    # gpsimd time at the head of the measured window.
    _insts = nc.m.functions[0].blocks[0].instructions
    for _i in range(len(_insts) - 1, -1, -1):
        if type(_insts[_i]).__name__ == "InstMemset":
            del _insts[_i]
    pk = nc.declare_dram_parameter("pack", [64, _F], fp32, isOutput=False)
    od = nc.declare_dram_parameter("out", [1, 4], fp32, isOutput=True)

    with tile.TileContext(nc) as tc:
        with (
            tc.tile_pool(name="sb", bufs=1) as sb,
            tc.tile_pool(name="ps", bufs=1, space="PSUM") as ps,
        ):
            P = sb.tile([64, _F], fp32)
            nc.sync.dma_start(out=P[:, :], in_=pk[:, :])

            PV = P.bitcast(bf16)  # (64, 2*_F) bf16 view of the same bytes
            _o = 2 * _C_BF
            W2 = PV[0:64, _o + _B_W2:_o + _B_W2 + 64]
            w3q = PV[0:32, _o + _B_W3:_o + _B_W3 + 1]
            w3s = PV[32:64, _o + _B_W3:_o + _B_W3 + 1]
            W1 = PV[0:16, _o + _B_W1:_o + _B_W1 + 64]
            XT = PV[0:16, _o + _B_XT:_o + _B_XT + 3]
            B1 = P[0:64, _C_B1:_C_B1 + 1]
            B2 = P[0:64, _C_B2:_C_B2 + 1]
            PA = P[0:3, _C_PA:_C_PA + 3]
            PB = P[0:3, _C_PB:_C_PB + 3]
            ON3 = P[0:3, _C_ON:_C_ON + 1]
            C3 = P[0:1, _C_C3:_C_C3 + 1]
            SB3 = P[0:1, _C_SB3:_C_SB3 + 1]

            # ACT primer: the ACTIVATE encoding supports a single sync wait,
            # but silu1 would need two (PE matmul + DMA'd bias tile). This
            # op makes ACT observe the input-DMA semaphore first, so silu1
            # only waits on PE.
            AJ = sb.tile([1, 1], fp32)
            nc.scalar.activation(AJ[:, :], C3, AF.Copy)

            # ---- pair-distance branch: rinv = 1/|p_i - p_j| per pair ----
            D = sb.tile([3, 3], fp32)
            nc.vector.tensor_sub(D[:, :], PA, PB)
            DD = sb.tile([3, 3], fp32)
            nc.gpsimd.tensor_mul(DD[:, :], D[:, :], D[:, :])
            R2p = ps.tile([1, 3], fp32)
            nc.tensor.matmul(R2p[:, :], ON3, DD[:, :], start=True, stop=True)
            # Quake rsqrt straight off PSUM: y0_bits = 0x5f3759df - (x>>1)
            #                              = ((x>>1) ^ -1) + 0x5f3759e0
            Y = sb.tile([1, 3], fp32)
            nc.vector.tensor_scalar(
                Y.bitcast(i32)[:, :], R2p.bitcast(i32)[:, :],
                1, -1, OP.arith_shift_right, OP.bitwise_xor,
            )
            nc.vector.tensor_scalar(
                Y.bitcast(i32)[:, :], Y.bitcast(i32)[:, :],
                0x5F3759E0, None, OP.add,
            )
            # Newton: y <- y * (1.5 - 0.5 x y^2); ~1.8e-3 rel after one step
            T = sb.tile([1, 3], fp32)
            nc.vector.tensor_mul(T[:, :], Y[:, :], Y[:, :])
            nc.vector.tensor_mul(T[:, :], T[:, :], R2p[:, :])
            nc.vector.tensor_scalar(T[:, :], T[:, :], -0.5, 1.5, OP.mult, OP.add)
            nc.vector.tensor_mul(Y[:, :], Y[:, :], T[:, :])

            # ---- fused dual MLP (hidden dims stacked: charge 0:32, sr 32:64) ----
            def silu(out_ap, in_psum, bias_ap, rows):
                if not silu_via_sigmoid:
                    nc.scalar.activation(out_ap, in_psum, AF.Silu, bias=bias_ap, scale=1.0)
                    return
                X1 = sb.tile([rows, 3], fp32, tag=f"x{rows}")
                nc.vector.tensor_scalar(X1[:, :], in_psum, bias_ap, None, OP.add)
                SG = sb.tile([rows, 3], fp32, tag=f"s{rows}")
                nc.scalar.activation(SG[:, :], in_psum, AF.Sigmoid, bias=bias_ap, scale=1.0)
                nc.vector.tensor_mul(out_ap, X1[:, :], SG[:, :])

            M1 = ps.tile([64, 3], fp32)
            nc.tensor.matmul(M1[:, :], W1, XT, start=True, stop=True)
            H1 = sb.tile([64, 3], bf16)
            silu(H1[:, :], M1[:, :], B1, 64)
            M2 = ps.tile([64, 3], fp32)
            nc.tensor.matmul(M2[:, :], W2, H1[:, :], start=True, stop=True)
            H2 = sb.tile([64, 3], bf16)
            silu(H2[:, :], M2[:, :], B2, 64)
            T3q = ps.tile([1, 3], fp32)  # per-atom raw_q (minus cb3, which cancels)
            nc.tensor.matmul(T3q[:, :], w3q, H2[0:32, :], start=True, stop=True)
            T3s = ps.tile([1, 3], fp32)  # per-atom short-range E (minus sb3)
            nc.tensor.matmul(T3s[:, :], w3s, H2[32:64, :], start=True, stop=True)

            # ---- charge correction + energy assembly ----
            # SR = sum_i T3s_i (3*sb3 is folded into the final add)
            SR = sb.tile([1, 1], fp32)
            nc.vector.reduce_sum(SR[:, :], T3s[:, :], axis=AX.X)
            # corr = -(sum T3q)/3 + C/3 in ONE fused op: out = T3q * -1/3,
            # accum = reduce_add(out) then + C/3 (scalar2 post-reduce)
            CR = sb.tile([1, 1], fp32)
            CAj = sb.tile([1, 3], fp32)
            nc.vector.tensor_scalar(
                CAj[:, :], T3q[:, :], -1.0 / 3.0, C3, OP.mult, OP.add,
                accum_out=CR[0:1, 0:1],
            )
            OUT = sb.tile([1, 4], fp32)
            nc.vector.scalar_tensor_tensor(
                OUT[0:1, 1:4], T3q[:, :], 1.0,
                CR[0:1, 0:1].broadcast_to((1, 3)), OP.mult, OP.add,
            )
            PQ = sb.tile([1, 3], fp32)
            nc.vector.tensor_mul(
                PQ[0:1, 0:2], OUT[0:1, 2:4],
                OUT[0:1, 1:2].broadcast_to((1, 2)),
            )
            nc.vector.tensor_mul(PQ[0:1, 2:3], OUT[0:1, 2:3], OUT[0:1, 3:4])
            # EE = sum(PQ * rinv), fused elementwise-mul + accumulate
            EPj = sb.tile([1, 3], fp32)
            EE = sb.tile([1, 1], fp32)
            nc.vector.scalar_tensor_tensor(
                EPj[:, :], PQ[:, :], 1.0, Y[:, :], OP.mult, OP.mult,
                accum_out=EE[0:1, 0:1],
            )
            nc.vector.scalar_tensor_tensor(
                OUT[0:1, 0:1], EE[:, :], SR[0:1, 0:1], SB3, OP.add, OP.add,
            )
            nc.sync.dma_start(out=od[:, :], in_=OUT[:, :])

    return nc


def _get_nc():
    global _NC
    if _NC is None:
        _NC = _build_nc()
    return _NC


def _to_bf16_words(a_f32):
    # (64, 2n) f32 -> bf16 (round-to-nearest-even) packed as (64, n) f32 words
    u = np.ascontiguousarray(a_f32, dtype=np.float32).view(np.uint32)
    b = ((u + 0x7FFF + ((u >> 16) & 1)) >> 16).astype(np.uint32)  # bf16 bits
    words = (b[:, 0::2] | (b[:, 1::2] << np.uint32(16))).astype(np.uint32)
    return words.view(np.float32)


def _pack_inputs(inputs):
    g = lambda k: np.asarray(inputs[k], dtype=np.float32)
    pack = np.zeros((64, _F), dtype=np.float32)
    pack[0:32, _C_B1] = g("cb1")
    pack[32:64, _C_B1] = g("sb1")
    pack[0:32, _C_B2] = g("cb2")
    pack[32:64, _C_B2] = g("sb2")
    pos = g("positions")
    pack[0:3, _C_PA:_C_PA + 3] = pos[list(_II)].T
    pack[0:3, _C_PB:_C_PB + 3] = pos[list(_JJ)].T
    pack[0:3, _C_ON] = 1.0
    pack[0, _C_C3] = float(np.asarray(inputs["charge_state"])) / 3.0
    pack[0, _C_SB3] = 3.0 * g("sb3")[0]

    packb = np.zeros((64, _FB), dtype=np.float32)
    packb[0:32, _B_W2:_B_W2 + 32] = g("cw2")
    packb[32:64, _B_W2 + 32:_B_W2 + 64] = g("sw2")
    packb[0:32, _B_W3] = g("cw3")[:, 0]
    packb[32:64, _B_W3] = g("sw3")[:, 0]
    packb[0:16, _B_W1:_B_W1 + 32] = g("cw1")
    packb[0:16, _B_W1 + 32:_B_W1 + 64] = g("sw1")
    packb[0:16, _B_XT:_B_XT + 3] = g("features").T
    pack[:, _C_BF:_C_BF + _FB // 2] = _to_bf16_words(packb)
    return pack


def _install_ntff_hook():
    # The image's antenv lacks axon_hooks; inject it and wire the ctypes
    # NTFF profile hook from trn_boot so trace=True yields exec_time_ns.
    try:
        from antenv.axon_hooks import get_axon_ntff_profile_hook  # noqa: F401
        return
    except ImportError:
        pass
    import types

    import antenv
    from trn_agent_boot.trn_boot import _ntff_profile_via_ctypes

    mod = types.ModuleType("antenv.axon_hooks")
    state = {"hook": None}
    mod.set_axon_ntff_profile_hook = lambda h: state.__setitem__("hook", h)
    mod.get_axon_ntff_profile_hook = lambda: state["hook"]
    sys.modules["antenv.axon_hooks"] = mod
    antenv.axon_hooks = mod
    mod.set_axon_ntff_profile_hook(
        _ntff_profile_via_ctypes("/opt/axon/libaxon_pjrt.so")
    )


def run(inputs, trace=False):
    """Run on hardware across 8 cores (replicated). Returns ((E, q), exec_ns)."""
    from concourse.bass_utils import run_bass_kernel_spmd

    if trace:
        _install_ntff_hook()
    nc = _get_nc()
    pack = _pack_inputs(inputs)
    in_maps = [{"pack": pack} for _ in range(8)]
    res = run_bass_kernel_spmd(nc, in_maps, list(range(8)), trace=trace)
    out = np.asarray(res.results[0]["out"], dtype=np.float32)
    E = np.asarray(out[0, 0], dtype=np.float32)
    q = out[0, 1:4].astype(np.float32)
    return (E, q), res.exec_time_ns


def kernel(**inputs):
    (E, q), _ = run(inputs)
    return (E, q)
